# revision 7
# baseline (speedup 1.0000x reference)
"""ExternalAttention Trainium2 Bass kernel (v2: transposed bf16 dataflow).

Math (per batch b, with N = H*W = 4096 tokens, C = 512, K = 64):
    x      = inputs @ w1 + b1          [N, C]
    logits = x @ m0                    [N, K]
    attn   = softmax(logits, axis=N)   (trailing L1-norm divides by 1+1e-9 ->
                                        skipped; max-shift skipped: logits O(1))
    y      = attn @ m1 @ w2            [N, C]
    out    = relu(BN_affine(y) + inputs)

Host-side folds (tiny C x C / C x K matrices):
    wm    = [w1 @ m0 | 0]                       [C, K+1]  (b1 @ m0 is constant
            per softmax column -> softmax-invariant, dropped; the zero column
            gives exp(0)=1, a ones-row that injects the BN shift via mm2)
    scale = gamma / sqrt(bn_var + eps); shift = beta - bn_mean * scale
    w2m   = [m1 @ (w2 * scale) ; shift]         [K+1, C]
    => out = relu(colsoftmax(x @ wm) @ w2m + x)

Device dataflow (per core, 2 batches, data-parallel over B=16 on 8 cores).
The host ships x TRANSPOSED per batch (xT [C, N], bf16) so the kernel never
transposes on the PE; everything runs in [c, n] layout and the host
transposes the bf16 output back.
    - mm1:  logitsT[65, n512] = sum_ct wm[ct]^T @ xT[ct]   (4 accumulating
      bf16 matmuls per 512-token chunk)
    - ACT exp psum -> attn [65, N] bf16 with per-chunk accumulated row sums;
      softmax row scale 1/sum is folded into a per-batch bf16 copy of w2m
      (scaling w2m rows == scaling attn rows), so attn itself is never
      rescaled
    - mm2:  deltaT[ct] [128, n512] = w2m_b[:, ct]^T @ attn  (single
      non-accumulating bf16 matmul, constant stationary per ctile)
    - residual add (psum fp32 + xT bf16 -> bf16) on DVE/Pool, relu in place
      on ACT/DVE/Pool, 1MB bf16 stores per ctile
    - two-batch software pipeline: b1's mm1 chunks interleave with b0's mm2
      so the PE never waits on b1's softmax reduction
"""

import os
import sys
from contextlib import ExitStack

import numpy as np
import ml_dtypes

for _p in ("/opt/trn_rl_repo", os.path.expanduser("~/.axon_site/_ro/trn_rl_repo")):
    if os.path.isdir(_p) and _p not in sys.path:
        sys.path.insert(0, _p)

import concourse.bass as bass
import concourse.mybir as mybir
import concourse.tile as tile
from concourse import bacc
from concourse.bass import ts
from concourse.bass_utils import run_bass_kernel_spmd

B, H, W, C, K = 16, 64, 64, 512, 64
N = H * W  # 4096 tokens
BN_EPS = 1e-3
NCORES = 8
BPC = B // NCORES  # batches per core = 2

F32 = mybir.dt.float32
BF16 = mybir.dt.bfloat16
BF16_NP = ml_dtypes.bfloat16

CT = 4          # channel tiles of 128
NQ = 8          # token chunks of 512 per batch
QW = 512        # chunk width
NLOAD = 4       # load slices per ctile (granularity NQ//NLOAD chunks each)

_cached_nc = None


def _build_nc() -> bass.Bass:
    nc = bacc.Bacc(None, target_bir_lowering=False, debug=False)
    x = nc.dram_tensor("x", [BPC, C, N], BF16, kind="ExternalInput")
    wm = nc.dram_tensor("wm", [C, K + 1], BF16, kind="ExternalInput")
    w2m = nc.dram_tensor("w2m", [K + 1, C], F32, kind="ExternalInput")
    ident = nc.dram_tensor("ident", [128, 128], BF16, kind="ExternalInput")
    y = nc.dram_tensor("y", [BPC, C, N], BF16, kind="ExternalOutput")

    with tile.TileContext(nc) as tc, ExitStack() as ctx:
        const = ctx.enter_context(tc.tile_pool(name="const", bufs=1))
        xt_pool = ctx.enter_context(tc.tile_pool(name="xt", bufs=2 * CT))
        attn_pool = ctx.enter_context(tc.tile_pool(name="attn", bufs=2))
        out_pool = ctx.enter_context(tc.tile_pool(name="out", bufs=2))
        small = ctx.enter_context(tc.tile_pool(name="small", bufs=4))

        wm_sb = const.tile([128, CT, K + 1], BF16)   # wm[ct*128+p, k]
        w2m_sb = const.tile([K + 1, C], F32)         # fp32 master
        ident_sb = const.tile([128, 128], BF16)

        xbs = [x[b].rearrange("(ct p) n -> ct p n", p=128) for b in range(BPC)]
        ybs = [y[b].rearrange("(ct p) n -> ct p n", p=128) for b in range(BPC)]

        xts, attns, sums_t, w2mb_t = [], [], [], []

        def load_batch(b, first=False):
            xts.append([xt_pool.tile([128, N], BF16, tag="xt",
                                     name=f"xt{b}_{ct}") for ct in range(CT)])
            lw = N // NLOAD
            for l in range(NLOAD):
                for ct in range(CT):
                    nc.sync.dma_start(out=xts[b][ct][:, ts(l, lw)],
                                      in_=xbs[b][ct][:, ts(l, lw)])
                if first and l == 0:
                    # constants ride behind the very first x slices
                    nc.sync.dma_start(
                        out=wm_sb,
                        in_=wm.rearrange("(ct p) k -> p ct k", p=128))
                    nc.sync.dma_start(out=w2m_sb, in_=w2m[:, :])
                    nc.sync.dma_start(out=ident_sb, in_=ident[:, :])
            attns.append(attn_pool.tile([K + 1, N], BF16, tag="attn",
                                        name=f"attn{b}"))
            sums_t.append(small.tile([K + 1, NQ], F32, tag="sums",
                                     name=f"sums{b}"))
            w2mb_t.append(small.tile([K + 1, C], BF16, tag="w2mb",
                                     name=f"w2mb{b}"))

        def mm1_chunk(l_psum, b, q):
            p_l = l_psum.tile([K + 1, QW], F32, tag="l")
            for ct in range(CT):
                nc.tensor.matmul(
                    p_l,
                    lhsT=wm_sb[:, ct],
                    rhs=xts[b][ct][:, ts(q, QW)],
                    start=(ct == 0),
                    stop=(ct == CT - 1),
                )
            # exp straight from psum; row K is exp(0)=1 (ones row);
            # per-chunk row sums accumulate into sums[:, q]
            nc.scalar.activation(
                out=attns[b][:, ts(q, QW)], in_=p_l,
                func=mybir.ActivationFunctionType.Exp,
                accum_out=sums_t[b][:, q:q + 1],
            )

        def softmax_finish(b):
            total = small.tile([K + 1, 1], F32, tag="total")
            nc.vector.reduce_sum(out=total, in_=sums_t[b],
                                 axis=mybir.AxisListType.X)
            rsum = small.tile([K + 1, 1], F32, tag="rsum")
            nc.vector.reciprocal(out=rsum, in_=total)
            # ones-row (BN shift) must not be normalized
            nc.vector.memset(rsum[K:K + 1], 1.0)
            # fold softmax 1/sum into the mm2 weights: w2m_b = rsum * w2m
            nc.vector.tensor_scalar_mul(w2mb_t[b], w2m_sb, rsum)

        def mm2_ctile(y_psum, b, ct):
            out_t = out_pool.tile([128, N], BF16, tag="out",
                                  name=f"out{b}_{ct}")
            for q in range(NQ):
                p_y = y_psum.tile([128, QW], F32, tag="y")
                pe_add = q % 2 == 0
                nc.tensor.matmul(
                    p_y,
                    lhsT=w2mb_t[b][:, ts(ct, 128)],
                    rhs=attns[b][:, ts(q, QW)],
                    start=True, stop=not pe_add,
                )
                if pe_add:
                    # residual add for free on the PE: psum += I @ xT
                    nc.tensor.matmul(
                        p_y,
                        lhsT=ident_sb,
                        rhs=xts[b][ct][:, ts(q, QW)],
                        start=False, stop=True,
                    )
                    if q == 6:
                        nc.vector.tensor_scalar_max(
                            out_t[:, ts(q, QW)], p_y, 0.0)
                    else:
                        nc.scalar.activation(
                            out=out_t[:, ts(q, QW)], in_=p_y,
                            func=mybir.ActivationFunctionType.Relu,
                        )
                else:
                    # DVE residual add (gpsimd can't read psum), gpsimd relu
                    nc.vector.scalar_tensor_tensor(
                        out=out_t[:, ts(q, QW)],
                        in0=p_y, scalar=1.0, in1=xts[b][ct][:, ts(q, QW)],
                        op0=mybir.AluOpType.mult, op1=mybir.AluOpType.add,
                    )
                    nc.gpsimd.tensor_scalar_max(
                        out_t[:, ts(q, QW)], out_t[:, ts(q, QW)], 0.0)
            nc.scalar.dma_start(out=ybs[b][ct], in_=out_t)

        load_batch(0, first=True)
        load_batch(1)

        with tc.tile_pool(name="lps", bufs=2, space="PSUM") as l_psum, \
             tc.tile_pool(name="yps", bufs=4, space="PSUM") as y_psum:
            for q in range(NQ):
                mm1_chunk(l_psum, 0, q)
            softmax_finish(0)
            # b1 mm1 interleaved with b0's mm2 epilogue
            for ct in range(CT):
                mm1_chunk(l_psum, 1, 2 * ct)
                mm1_chunk(l_psum, 1, 2 * ct + 1)
                if ct == CT - 1:
                    softmax_finish(1)
                mm2_ctile(y_psum, 0, ct)
            for ct in range(CT):
                mm2_ctile(y_psum, 1, ct)

    nc.finalize()
    return nc


def _get_nc() -> bass.Bass:
    global _cached_nc
    if _cached_nc is None:
        _cached_nc = _build_nc()
    return _cached_nc


def _fold_weights(w1, m0, m1, w2, gamma, beta, bn_mean, bn_var):
    w1 = np.asarray(w1, np.float64)
    m0 = np.asarray(m0, np.float64)
    m1 = np.asarray(m1, np.float64)
    w2 = np.asarray(w2, np.float64)
    gamma = np.asarray(gamma, np.float64)
    beta = np.asarray(beta, np.float64)
    bn_mean = np.asarray(bn_mean, np.float64)
    bn_var = np.asarray(bn_var, np.float64)

    wm_aug = np.zeros((C, K + 1), np.float32)
    wm_aug[:, :K] = (w1 @ m0).astype(np.float32)  # col K stays 0 -> ones row
    scale = gamma / np.sqrt(bn_var + BN_EPS)
    w2m_aug = np.zeros((K + 1, C), np.float32)
    w2m_aug[:K] = (m1 @ (w2 * scale[None, :])).astype(np.float32)
    w2m_aug[K] = (beta - bn_mean * scale).astype(np.float32)  # shift row
    return wm_aug, w2m_aug


def _run(inputs_np: dict, trace: bool = False):
    nc = _get_nc()
    inp = np.asarray(inputs_np["inputs"], np.float32)
    wm_aug, w2m_aug = _fold_weights(
        inputs_np["w1"], inputs_np["m0"], inputs_np["m1"], inputs_np["w2"],
        inputs_np["gamma"], inputs_np["beta"],
        inputs_np["bn_mean"], inputs_np["bn_var"],
    )
    # per-batch transposed bf16 inputs: [B, C, N]
    xT = np.ascontiguousarray(
        inp.reshape(B, N, C).astype(BF16_NP).transpose(0, 2, 1))
    wm_bf = wm_aug.astype(BF16_NP)
    eye = np.eye(128, dtype=BF16_NP)
    in_maps = [
        {
            "x": xT[i * BPC:(i + 1) * BPC],
            "wm": wm_bf,
            "w2m": w2m_aug,
            "ident": eye,
        }
        for i in range(NCORES)
    ]
    res = run_bass_kernel_spmd(nc, in_maps, core_ids=list(range(NCORES)),
                               trace=trace)
    out = np.concatenate([r["y"] for r in res.results], axis=0)  # [B, C, N]
    out = out.transpose(0, 2, 1).astype(np.float32)
    return np.ascontiguousarray(out).reshape(B, H, W, C), res


def kernel(**inputs) -> np.ndarray:
    out, _ = _run(inputs, trace=False)
    return out


# revision 9
# speedup vs baseline: 3.2155x; 3.2155x over previous
"""ExternalAttention Trainium2 Bass kernel (v2: transposed bf16 dataflow).

Math (per batch b, with N = H*W = 4096 tokens, C = 512, K = 64):
    x      = inputs @ w1 + b1          [N, C]
    logits = x @ m0                    [N, K]
    attn   = softmax(logits, axis=N)   (trailing L1-norm divides by 1+1e-9 ->
                                        skipped; max-shift skipped: logits O(1))
    y      = attn @ m1 @ w2            [N, C]
    out    = relu(BN_affine(y) + inputs)

Host-side folds (tiny C x C / C x K matrices):
    wm    = [w1 @ m0 | 0]                       [C, K+1]  (b1 @ m0 is constant
            per softmax column -> softmax-invariant, dropped; the zero column
            gives exp(0)=1, a ones-row that injects the BN shift via mm2)
    scale = gamma / sqrt(bn_var + eps); shift = beta - bn_mean * scale
    w2m   = [m1 @ (w2 * scale) ; shift]         [K+1, C]
    => out = relu(colsoftmax(x @ wm) @ w2m + x)

Device dataflow (per core, 2 batches, data-parallel over B=16 on 8 cores).
The host ships x TRANSPOSED per batch (xT [C, N], bf16) so the kernel never
transposes on the PE; everything runs in [c, n] layout and the host
transposes the bf16 output back.
    - mm1:  logitsT[65, n512] = sum_ct wm[ct]^T @ xT[ct]   (4 accumulating
      bf16 matmuls per 512-token chunk)
    - ACT exp psum -> attn [65, N] bf16 with per-chunk accumulated row sums;
      softmax row scale 1/sum is folded into a per-batch bf16 copy of w2m
      (scaling w2m rows == scaling attn rows), so attn itself is never
      rescaled
    - mm2:  deltaT[ct] [128, n512] = w2m_b[:, ct]^T @ attn  (single
      non-accumulating bf16 matmul, constant stationary per ctile)
    - residual add (psum fp32 + xT bf16 -> bf16) on DVE/Pool, relu in place
      on ACT/DVE/Pool, 1MB bf16 stores per ctile
    - two-batch software pipeline: b1's mm1 chunks interleave with b0's mm2
      so the PE never waits on b1's softmax reduction
"""

import os
import sys
from contextlib import ExitStack

import numpy as np
import ml_dtypes

for _p in ("/opt/trn_rl_repo", os.path.expanduser("~/.axon_site/_ro/trn_rl_repo")):
    if os.path.isdir(_p) and _p not in sys.path:
        sys.path.insert(0, _p)

import concourse.bass as bass
import concourse.mybir as mybir
import concourse.tile as tile
from concourse import bacc
from concourse.bass import ts
from concourse.bass_utils import run_bass_kernel_spmd

B, H, W, C, K = 16, 64, 64, 512, 64
N = H * W  # 4096 tokens
BN_EPS = 1e-3
NCORES = 8
BPC = B // NCORES  # batches per core = 2

F32 = mybir.dt.float32
BF16 = mybir.dt.bfloat16
BF16_NP = ml_dtypes.bfloat16

CT = 4          # channel tiles of 128
NQ = 8          # token chunks of 512 per batch
QW = 512        # chunk width
NLOAD = 4       # load slices per ctile (granularity NQ//NLOAD chunks each)

_cached_nc = None


def _build_nc() -> bass.Bass:
    nc = bacc.Bacc(None, target_bir_lowering=False, debug=False)
    x = nc.dram_tensor("x", [BPC, C, N], BF16, kind="ExternalInput")
    wm = nc.dram_tensor("wm", [C, K + 1], BF16, kind="ExternalInput")
    w2m = nc.dram_tensor("w2m", [K + 1, C], F32, kind="ExternalInput")
    ident = nc.dram_tensor("ident", [128, 128], BF16, kind="ExternalInput")
    y = nc.dram_tensor("y", [BPC, C, N], BF16, kind="ExternalOutput")

    with tile.TileContext(nc) as tc, ExitStack() as ctx:
        const = ctx.enter_context(tc.tile_pool(name="const", bufs=1))
        xt_pool = ctx.enter_context(tc.tile_pool(name="xt", bufs=2 * CT))
        attn_pool = ctx.enter_context(tc.tile_pool(name="attn", bufs=2))
        out_pool = ctx.enter_context(tc.tile_pool(name="out", bufs=2))
        small = ctx.enter_context(tc.tile_pool(name="small", bufs=4))

        wm_sb = const.tile([128, CT, K + 1], BF16)   # wm[ct*128+p, k]
        w2m_sb = const.tile([K + 1, C], F32)         # fp32 master
        ident_sb = const.tile([128, 128], BF16)

        xbs = [x[b].rearrange("(ct p) n -> ct p n", p=128) for b in range(BPC)]
        ybs = [y[b].rearrange("(ct p) n -> ct p n", p=128) for b in range(BPC)]

        xts, attns, sums_t, w2mb_t = [], [], [], []

        def load_batch(b, first=False):
            xts.append([xt_pool.tile([128, N], BF16, tag="xt",
                                     name=f"xt{b}_{ct}") for ct in range(CT)])
            lw = N // NLOAD
            for l in range(NLOAD):
                for ct in range(CT):
                    nc.sync.dma_start(out=xts[b][ct][:, ts(l, lw)],
                                      in_=xbs[b][ct][:, ts(l, lw)])
                if first and l == 0:
                    # constants ride behind the very first x slices
                    nc.sync.dma_start(
                        out=wm_sb,
                        in_=wm.rearrange("(ct p) k -> p ct k", p=128))
                    nc.sync.dma_start(out=w2m_sb, in_=w2m[:, :])
                    nc.sync.dma_start(out=ident_sb, in_=ident[:, :])
            attns.append(attn_pool.tile([K + 1, N], BF16, tag="attn",
                                        name=f"attn{b}"))
            sums_t.append(small.tile([K + 1, NQ], F32, tag="sums",
                                     name=f"sums{b}"))
            w2mb_t.append(small.tile([K + 1, C], BF16, tag="w2mb",
                                     name=f"w2mb{b}"))

        def mm1_chunk(l_psum, b, q):
            p_l = l_psum.tile([K + 1, QW], F32, tag="l")
            for ct in range(CT):
                nc.tensor.matmul(
                    p_l,
                    lhsT=wm_sb[:, ct],
                    rhs=xts[b][ct][:, ts(q, QW)],
                    start=(ct == 0),
                    stop=(ct == CT - 1),
                )
            # exp straight from psum; row K is exp(0)=1 (ones row);
            # per-chunk row sums accumulate into sums[:, q]
            nc.scalar.activation(
                out=attns[b][:, ts(q, QW)], in_=p_l,
                func=mybir.ActivationFunctionType.Exp,
                accum_out=sums_t[b][:, q:q + 1],
            )

        def softmax_finish(b):
            total = small.tile([K + 1, 1], F32, tag="total")
            nc.vector.reduce_sum(out=total, in_=sums_t[b],
                                 axis=mybir.AxisListType.X)
            rsum = small.tile([K + 1, 1], F32, tag="rsum")
            nc.vector.reciprocal(out=rsum, in_=total)
            # ones-row (BN shift) must not be normalized
            nc.vector.memset(rsum[K:K + 1], 1.0)
            # fold softmax 1/sum into the mm2 weights: w2m_b = rsum * w2m
            # (ACT per-partition scale; DVE's tensor_scalar here costs 3us)
            nc.scalar.activation(
                out=w2mb_t[b], in_=w2m_sb,
                func=mybir.ActivationFunctionType.Copy, scale=rsum)

        def mm2_ctile(y_psum, b, ct):
            out_t = out_pool.tile([128, N], BF16, tag="out",
                                  name=f"out{b}_{ct}")
            for q in range(NQ):
                p_y = y_psum.tile([128, QW], F32, tag="y")
                pe_add = q < 5
                nc.tensor.matmul(
                    p_y,
                    lhsT=w2mb_t[b][:, ts(ct, 128)],
                    rhs=attns[b][:, ts(q, QW)],
                    start=True, stop=not pe_add,
                )
                if pe_add:
                    # residual add for free on the PE: psum += I @ xT;
                    # relu straight from psum on ACT
                    nc.tensor.matmul(
                        p_y,
                        lhsT=ident_sb,
                        rhs=xts[b][ct][:, ts(q, QW)],
                        start=False, stop=True,
                    )
                    nc.scalar.activation(
                        out=out_t[:, ts(q, QW)], in_=p_y,
                        func=mybir.ActivationFunctionType.Relu,
                    )
                else:
                    # DVE residual add psum+xT -> bf16, then cheap 16-bit
                    # in-place relu on DVE
                    nc.vector.scalar_tensor_tensor(
                        out=out_t[:, ts(q, QW)],
                        in0=p_y, scalar=1.0, in1=xts[b][ct][:, ts(q, QW)],
                        op0=mybir.AluOpType.mult, op1=mybir.AluOpType.add,
                    )
                    nc.vector.tensor_scalar_max(
                        out_t[:, ts(q, QW)], out_t[:, ts(q, QW)], 0.0)
            nc.scalar.dma_start(out=ybs[b][ct], in_=out_t)

        load_batch(0, first=True)
        load_batch(1)

        with tc.tile_pool(name="lps", bufs=2, space="PSUM") as l_psum, \
             tc.tile_pool(name="yps", bufs=4, space="PSUM") as y_psum:
            for q in range(NQ):
                mm1_chunk(l_psum, 0, q)
            softmax_finish(0)
            # b1 mm1 interleaved with b0's mm2 epilogue
            for ct in range(CT):
                mm1_chunk(l_psum, 1, 2 * ct)
                mm1_chunk(l_psum, 1, 2 * ct + 1)
                if ct == CT - 1:
                    softmax_finish(1)
                mm2_ctile(y_psum, 0, ct)
            for ct in range(CT):
                mm2_ctile(y_psum, 1, ct)

    nc.finalize()
    return nc


def _get_nc() -> bass.Bass:
    global _cached_nc
    if _cached_nc is None:
        _cached_nc = _build_nc()
    return _cached_nc


def _fold_weights(w1, m0, m1, w2, gamma, beta, bn_mean, bn_var):
    w1 = np.asarray(w1, np.float64)
    m0 = np.asarray(m0, np.float64)
    m1 = np.asarray(m1, np.float64)
    w2 = np.asarray(w2, np.float64)
    gamma = np.asarray(gamma, np.float64)
    beta = np.asarray(beta, np.float64)
    bn_mean = np.asarray(bn_mean, np.float64)
    bn_var = np.asarray(bn_var, np.float64)

    wm_aug = np.zeros((C, K + 1), np.float32)
    wm_aug[:, :K] = (w1 @ m0).astype(np.float32)  # col K stays 0 -> ones row
    scale = gamma / np.sqrt(bn_var + BN_EPS)
    w2m_aug = np.zeros((K + 1, C), np.float32)
    w2m_aug[:K] = (m1 @ (w2 * scale[None, :])).astype(np.float32)
    w2m_aug[K] = (beta - bn_mean * scale).astype(np.float32)  # shift row
    return wm_aug, w2m_aug


def _run(inputs_np: dict, trace: bool = False):
    nc = _get_nc()
    inp = np.asarray(inputs_np["inputs"], np.float32)
    wm_aug, w2m_aug = _fold_weights(
        inputs_np["w1"], inputs_np["m0"], inputs_np["m1"], inputs_np["w2"],
        inputs_np["gamma"], inputs_np["beta"],
        inputs_np["bn_mean"], inputs_np["bn_var"],
    )
    # per-batch transposed bf16 inputs: [B, C, N]
    xT = np.ascontiguousarray(
        inp.reshape(B, N, C).astype(BF16_NP).transpose(0, 2, 1))
    wm_bf = wm_aug.astype(BF16_NP)
    eye = np.eye(128, dtype=BF16_NP)
    in_maps = [
        {
            "x": xT[i * BPC:(i + 1) * BPC],
            "wm": wm_bf,
            "w2m": w2m_aug,
            "ident": eye,
        }
        for i in range(NCORES)
    ]
    res = run_bass_kernel_spmd(nc, in_maps, core_ids=list(range(NCORES)),
                               trace=trace)
    out = np.concatenate([r["y"] for r in res.results], axis=0)  # [B, C, N]
    out = out.transpose(0, 2, 1).astype(np.float32)
    return np.ascontiguousarray(out).reshape(B, H, W, C), res


def kernel(**inputs) -> np.ndarray:
    out, _ = _run(inputs, trace=False)
    return out
